# revision 1
# baseline (speedup 1.0000x reference)
"""Trainium2 Bass kernel: per-batch segment-mean pooling + 3-layer MLP.

Reference computation (B=64, T=512, H=768, S=128):
  pooled[b,s,:] = mean over t of hidden[b,t,:] where statements_ids[b,t]==s
  x = gelu(pooled @ w1 + b1); x = gelu(x @ w2 + b2)
  out[b,s] = sigmoid(x @ w3 + b3)

Distribution: data-parallel over batch across 8 NeuronCores (8 batches per
core); MLP weights replicated.

Per-core algorithm (all matmuls on PE at fp32r 1 cycle/row):
  - Build the one-hot matrix MT[t,s] = (sid[t]==s) on DVE via
    tensor_scalar(is_equal) against an iota constant.
  - counts = MT.T @ ones            (PE)        -> inv = 1/max(counts,1) (DVE)
  - pooled_sums = MT.T @ hidden[b]  (PE, [S,H]) -> pooled = sums*inv     (DVE)
  - X^T tiles via PE transpose (pooled is [S,H] but the MLP wants [H, rows])
  - MLP batched over all 8 local batches: rows = 8*128 = 1024 moving dim,
    weights stationary; gelu/sigmoid + bias fused on ACT.
"""

import os
import sys

sys.path.insert(0, "/opt/trn_rl_repo")

import numpy as np

import concourse.bass as bass
import concourse.mybir as mybir
import concourse.tile as tile
from concourse import bacc, bass_utils

B, T, H, S = 64, 512, 768, 128
N_CORES = 8
BL = B // N_CORES  # local batches per core
P = 128
KT = T // P        # t-tiles per batch
KH = H // P        # h-tiles
R = BL * S         # MLP rows per core
RC = 2 * S         # moving-dim chunk (2 batches) -- >=256 keeps fp32r at 1 cyc/row
NRC = R // RC
HF = H + 2         # hidden padded with 2 constant 1.0 columns (counts trick)
CR_COLS = 134      # f32r packed consts (matmul operands): ident | w3
CF_COLS = 173      # f32 packed consts: iota | sid-bits | b1 | b2 | b3

_CACHE: dict = {}


def _build_program(act_func=None):
    f32, f32r, i32 = mybir.dt.float32, mybir.dt.float32r, mybir.dt.int32
    FT = mybir.ActivationFunctionType
    OP = mybir.AluOpType

    nc = bacc.Bacc("TRN2", target_bir_lowering=False, debug=False)
    hid = nc.dram_tensor("hidden", [BL, T, HF], f32r, kind="ExternalInput").ap()
    w1 = nc.dram_tensor("w1", [H, H], f32r, kind="ExternalInput").ap()
    w2 = nc.dram_tensor("w2", [H, H], f32r, kind="ExternalInput").ap()
    cpack_r = nc.dram_tensor("cpack_r", [P, CR_COLS], f32r, kind="ExternalInput").ap()
    cpack_f = nc.dram_tensor("cpack_f", [P, CF_COLS], f32, kind="ExternalInput").ap()
    out = nc.dram_tensor("out", [BL, S], f32, kind="ExternalOutput").ap()

    with tile.TileContext(nc) as tc:
        with (
            tc.tile_pool(name="consts", bufs=1) as consts,
            tc.tile_pool(name="wpool", bufs=1) as wpool,
            tc.tile_pool(name="hpool", bufs=1) as hpool,
            tc.tile_pool(name="mtpool", bufs=8) as mtpool,
            tc.tile_pool(name="small", bufs=3) as small,
            tc.tile_pool(name="xtpool", bufs=1) as xtpool,
            tc.tile_pool(name="ypool", bufs=1) as ypool,
            tc.tile_pool(name="ps", bufs=8, space="PSUM") as ps,
        ):
            # ---- all small constants arrive in ONE packed DMA (single
            # 1.2KB line per partition) so the hidden stream starts at once ----
            cpf_sb = consts.tile([P, CF_COLS], f32)
            nc.sync.dma_start(cpf_sb, cpack_f)
            cpr_sb = consts.tile([P, CR_COLS], f32r)
            nc.sync.dma_start(cpr_sb, cpack_r)
            ident_sb = cpr_sb[:, 0:P]
            w3_sb = cpr_sb[:, P : P + KH]
            iota_sb = cpf_sb[:, 0:P]
            sid_sb = cpf_sb[:, P : P + BL * KT].bitcast(i32)
            b1_sb = cpf_sb[:, 160:166]
            b2_sb = cpf_sb[:, 166:172]
            b3_sb = cpf_sb[0:1, 172:173]

            # ---- hidden + weight streaming on sync/HWDGE, ordered to match
            # the compute pipeline: hidden batches pace the pooling; weight
            # k-tiles trickle between batches so fc1/fc2 unlock per-k ----
            hbs = [None] * BL
            w1ks = [None] * KH
            w2ks = [None] * KH

            def load_hb(b):
                if b < 2:
                    # first two batches arrive per k-chunk so pooling starts
                    # on the first 0.4 MB instead of the full 1.6 MB batch
                    tiles = []
                    for k in range(KT):
                        t = hpool.tile([P, HF], f32r, tag=f"hb{b}k{k}", name=f"hb{b}k{k}")
                        nc.sync.dma_start(t, hid[b, k * P : (k + 1) * P, :])
                        tiles.append(t)
                    hbs[b] = tiles
                else:
                    hb = hpool.tile(
                        [P, KT, HF], f32r, tag=f"hb{2 + (b - 2) % 3}", name=f"hb{b}"
                    )
                    nc.sync.dma_start(hb, hid[b].rearrange("(k p) h -> p k h", p=P))
                    hbs[b] = hb

            def hb_slice(b, k, lo, hi):
                if b < 2:
                    return hbs[b][k][:, lo:hi]
                return hbs[b][:, k, lo:hi]

            def load_w(ws, wdram, k, nm):
                ws[k] = wpool.tile([P, H], f32r, tag=f"{nm}{k}", name=f"{nm}{k}")
                nc.sync.dma_start(ws[k], wdram[k * P : (k + 1) * P, :])

            load_hb(0)
            for k in range(3):
                load_w(w1ks, w1, k, "w1k")
            load_hb(1)
            for k in range(3, KH):
                load_w(w1ks, w1, k, "w1k")
            load_hb(2)
            load_hb(3)
            for k in range(KH):
                load_w(w2ks, w2, k, "w2k")
            load_hb(4)
            load_hb(5)
            load_hb(6)
            load_hb(7)

            xts = [xtpool.tile([P, R], f32r, tag=f"xt{k}", name=f"xt{k}") for k in range(KH)]
            y1s = [ypool.tile([P, R], f32r, tag=f"y1_{m}", name=f"y1_{m}") for m in range(KH)]
            y2s = [ypool.tile([P, R], f32r, tag=f"y2_{m}", name=f"y2_{m}") for m in range(KH)]
            pred = ypool.tile([1, R], f32, tag="pred")

            C0 = 512          # pooling psum chunk 0: cols [0, 512)
            C1 = HF - C0      # chunk 1: cols [512, 770) -- col 768 = counts

            pooleds = [None] * BL

            def pool_mm(b):
                sidf = small.tile([P, KT], f32, tag="sidf")
                nc.vector.tensor_copy(sidf, sid_sb[:, b * KT : (b + 1) * KT])
                mts = []
                for k in range(KT):
                    mt = mtpool.tile([P, P], f32r, tag="mt")
                    nc.vector.tensor_tensor(
                        mt,
                        iota_sb,
                        sidf[:, k : k + 1].to_broadcast((P, P)),
                        OP.is_equal,
                    )
                    mts.append(mt)
                # counts chunk first so the inv chain runs while pp0 matmuls
                pp1 = ps.tile([P, C1], f32, tag="ps")
                pp0 = ps.tile([P, C0], f32, tag="ps")
                # interleave the two accumulation groups per k-chunk: both
                # matmuls of an arrived chunk fire at once instead of pp1(k3)
                # blocking ready pp0 work in the in-order PE stream
                for k in range(KT):
                    nc.tensor.matmul(
                        pp1, lhsT=mts[k], rhs=hb_slice(b, k, C0, HF),
                        start=(k == 0), stop=(k == KT - 1),
                    )
                    nc.tensor.matmul(
                        pp0, lhsT=mts[k], rhs=hb_slice(b, k, 0, C0),
                        start=(k == 0), stop=(k == KT - 1),
                    )
                inv = small.tile([P, 1], f32, tag="inv")
                nc.vector.tensor_scalar(inv, pp1[:, H - C0 : H - C0 + 1], 1.0, None, OP.max)
                nc.vector.reciprocal(inv, inv)
                pooled = small.tile([P, H], f32r, tag="pooled")
                # normalize in transpose-consumption order, smallest first:
                # [0:128] unblocks transpose m0 immediately, [128:512] covers
                # m1-m3 while m0 runs, [512:768] covers m4-m5
                nc.vector.tensor_tensor(
                    pooled[:, 0:P], pp0[:, 0:P], inv[:, 0:1].to_broadcast((P, P)),
                    OP.mult,
                )
                nc.vector.tensor_tensor(
                    pooled[:, P:C0], pp0[:, P:C0],
                    inv[:, 0:1].to_broadcast((P, C0 - P)), OP.mult,
                )
                nc.vector.tensor_tensor(
                    pooled[:, C0:H], pp1[:, 0 : H - C0],
                    inv[:, 0:1].to_broadcast((P, H - C0)), OP.mult,
                )
                pooleds[b] = pooled

            def pool_tr(b):
                pooled = pooleds[b]
                for m in range(KH):
                    trp = ps.tile([P, P], f32r, tag="ps")
                    nc.tensor.transpose(trp, pooled[:, m * P : (m + 1) * P], ident_sb)
                    nc.vector.tensor_copy(xts[m][:, b * S : (b + 1) * S], trp)

            def fc(wks, b_sb, xs, outs, rc, func):
                for m in range(KH):
                    pt = ps.tile([P, RC], f32, tag="ps")
                    for k in range(KH):
                        nc.tensor.matmul(
                            pt,
                            lhsT=wks[k][:, m * P : (m + 1) * P],
                            rhs=xs[k][:, rc * RC : (rc + 1) * RC],
                            start=(k == 0),
                            stop=(k == KH - 1),
                        )
                    nc.scalar.activation(
                        outs[m][:, rc * RC : (rc + 1) * RC],
                        pt,
                        func,
                        bias=b_sb[:, m : m + 1],
                    )

            def fc3(rc):
                pt = ps.tile([1, RC], f32, tag="ps")
                for k in range(KH):
                    nc.tensor.matmul(
                        pt,
                        lhsT=w3_sb[:, k : k + 1],
                        rhs=y2s[k][:, rc * RC : (rc + 1) * RC],
                        start=(k == 0),
                        stop=(k == KH - 1),
                    )
                nc.scalar.activation(
                    pred[:, rc * RC : (rc + 1) * RC],
                    pt,
                    mybir.ActivationFunctionType.Sigmoid,
                    bias=b3_sb,
                )
                # stream this chunk's predictions out immediately; only the
                # final 1 KB remains on the critical path after the last sigmoid
                nc.sync.dma_start(
                    out.rearrange("b s -> (b s)")[rc * RC : (rc + 1) * RC],
                    pred[:, rc * RC : (rc + 1) * RC],
                )

            FT = mybir.ActivationFunctionType
            gelu = FT.Gelu if act_func is None else act_func
            pool_mm(0)
            pool_tr(0)
            pool_mm(1)
            pool_tr(1)
            fc(w1ks, b1_sb, xts, y1s, 0, gelu)
            pool_mm(2)
            pool_tr(2)
            pool_mm(3)
            pool_tr(3)
            fc(w1ks, b1_sb, xts, y1s, 1, gelu)
            fc(w2ks, b2_sb, y1s, y2s, 0, gelu)
            fc3(0)
            pool_mm(4)
            pool_tr(4)
            pool_mm(5)
            pool_tr(5)
            fc(w1ks, b1_sb, xts, y1s, 2, gelu)
            fc(w2ks, b2_sb, y1s, y2s, 1, gelu)
            fc3(1)
            pool_mm(6)
            pool_tr(6)
            pool_mm(7)
            pool_tr(7)
            fc(w1ks, b1_sb, xts, y1s, 3, gelu)
            fc(w2ks, b2_sb, y1s, y2s, 2, gelu)
            fc3(2)
            fc(w2ks, b2_sb, y1s, y2s, 3, gelu)
            fc3(3)

    nc.compile()
    return nc


def _get_program():
    if "nc" not in _CACHE:
        _CACHE["nc"] = _build_program()
    return _CACHE["nc"]


def _cpack(sid_shard, b1, b2, b3, w3):
    """Pack per-core constants into two tensors: f32r (matmul operands,
    the DMA may round these) and plain f32 (bit-exact: iota, sid bits,
    biases)."""
    cr = np.zeros((P, CR_COLS), dtype=np.float32)
    cr[:, 0:P] = np.eye(P, dtype=np.float32)
    cr[:, P : P + KH] = np.asarray(w3, np.float32).reshape(KH, P, 1)[:, :, 0].T
    cf = np.zeros((P, CF_COLS), dtype=np.float32)
    cf[:, 0:P] = np.arange(P, dtype=np.float32)[None, :]
    sid_cols = np.transpose(
        sid_shard.astype(np.int32).reshape(BL, KT, P), (2, 0, 1)
    ).reshape(P, BL * KT)
    cf[:, P : P + BL * KT] = sid_cols.view(np.float32)
    cf[:, 160:166] = np.asarray(b1, np.float32).reshape(KH, P).T
    cf[:, 166:172] = np.asarray(b2, np.float32).reshape(KH, P).T
    cf[0, 172] = np.float32(np.asarray(b3).reshape(-1)[0])
    return cr, cf


def make_in_maps(hidden, statements_ids, w1, b1, w2, b2, w3, b3):
    hidden = np.asarray(hidden, dtype=np.float32)
    pad = np.ones((*hidden.shape[:2], HF - H), dtype=np.float32)
    hidden = np.ascontiguousarray(np.concatenate([hidden, pad], axis=-1))
    sid = np.asarray(statements_ids, dtype=np.int32)
    w1 = np.ascontiguousarray(np.asarray(w1, dtype=np.float32))
    w2 = np.ascontiguousarray(np.asarray(w2, dtype=np.float32))
    in_maps = []
    for c in range(N_CORES):
        cr, cf = _cpack(sid[c * BL : (c + 1) * BL], b1, b2, b3, w3)
        in_maps.append(
            {
                "hidden": hidden[c * BL : (c + 1) * BL],
                "w1": w1,
                "w2": w2,
                "cpack_r": cr,
                "cpack_f": cf,
            }
        )
    return in_maps


def kernel(hidden, statements_ids, w1, b1, w2, b2, w3, b3, **kwargs):
    nc = _get_program()
    in_maps = make_in_maps(hidden, statements_ids, w1, b1, w2, b2, w3, b3)
    trace = bool(int(os.environ.get("KERNEL_TRACE", "0")))
    res = bass_utils.run_bass_kernel_spmd(
        nc, in_maps, core_ids=list(range(N_CORES)), trace=trace
    )
    _CACHE["last_results"] = res
    out = np.concatenate([res.results[c]["out"] for c in range(N_CORES)], axis=0)
    return out.astype(np.float32)



# revision 7
# speedup vs baseline: 1.2496x; 1.2496x over previous
"""Trainium2 Bass kernel: per-batch segment-mean pooling + 3-layer MLP.

Reference computation (B=64, T=512, H=768, S=128):
  pooled[b,s,:] = mean over t of hidden[b,t,:] where statements_ids[b,t]==s
  x = gelu(pooled @ w1 + b1); x = gelu(x @ w2 + b2)
  out[b,s] = sigmoid(x @ w3 + b3)

Distribution: data-parallel over batch across 8 NeuronCores (8 batches per
core); MLP weights replicated.

Per-core algorithm (all matmuls bf16, 1 cyc/row on PE):
  - mt_scaled[t,s] = (sid[t]==s)/count[sid[t]] built in one fused DVE
    tensor_scalar (is_equal then mult); inverse counts precomputed on host.
  - pooledT[h,s] = hidden_tile[t,h].T @ mt_scaled[t,s]  (PE, accumulated
    over the 4 t-tiles) -> already in the MLP's [contraction-on-partition]
    layout, so no PE transposes at all.
  - PSUM->SBUF drains of pooledT split between DVE and GPSIMD.
  - MLP: weights stationary, activations moving; gelu/sigmoid + bias fused
    on ACT.  All gelus complete before any sigmoid so the activation table
    loads exactly twice (dummy ACTs prefetch each table off the critical
    path).
"""

import os
import sys

sys.path.insert(0, "/opt/trn_rl_repo")

import numpy as np
import ml_dtypes

import concourse.bass as bass
import concourse.mybir as mybir
import concourse.tile as tile
from concourse import bacc, bass_utils

B, T, H, S = 64, 512, 768, 128
N_CORES = 8
BL = B // N_CORES  # local batches per core
P = 128
KT = T // P        # t-tiles per batch
KH = H // P        # h-tiles
R = BL * S         # MLP rows per core (= 1024)
RC = 256           # fc1 moving-dim sub-chunk (2 batches)
RW = 512           # ACT drain / fc2 / fc3 chunk (4 batches)

# cpf (f32 packed consts) column layout
C_IOTA = 0                  # [128, 128] iota along free dim
C_SID = C_IOTA + P          # [128, 32]  sid, token-major per (b, k) column
C_ICNT = C_SID + BL * KT    # [128, 32]  1/count per token, same layout
C_B1 = C_ICNT + BL * KT     # [128, 6]
C_B2 = C_B1 + KH            # [128, 6]
C_B3 = C_B2 + KH            # [1] at row 0
CPF_COLS = C_B3 + 1

_CACHE: dict = {}


def _build_program():
    f32, bf16 = mybir.dt.float32, mybir.dt.bfloat16
    FT = mybir.ActivationFunctionType
    OP = mybir.AluOpType

    nc = bacc.Bacc("TRN2", target_bir_lowering=False, debug=False)
    hid = nc.dram_tensor("hidden", [BL, T, H], bf16, kind="ExternalInput").ap()
    w1 = nc.dram_tensor("w1", [H, H], bf16, kind="ExternalInput").ap()
    w2 = nc.dram_tensor("w2", [H, H], bf16, kind="ExternalInput").ap()
    w3p = nc.dram_tensor("w3p", [P, KH], bf16, kind="ExternalInput").ap()
    cpf = nc.dram_tensor("cpf", [P, CPF_COLS], f32, kind="ExternalInput").ap()
    out = nc.dram_tensor("out", [BL, S], f32, kind="ExternalOutput").ap()

    with tile.TileContext(nc) as tc:
        with (
            tc.tile_pool(name="consts", bufs=1) as consts,
            tc.tile_pool(name="wpool", bufs=1) as wpool,
            tc.tile_pool(name="hpool", bufs=1) as hpool,
            tc.tile_pool(name="mtpool", bufs=8) as mtpool,
            tc.tile_pool(name="xpool", bufs=1) as xpool,
            tc.tile_pool(name="pp", bufs=4, space="PSUM") as pp,
            tc.tile_pool(name="pf", bufs=2, space="PSUM") as pf,
            tc.tile_pool(name="p3", bufs=1, space="PSUM") as p3,
        ):
            # consts first on the sync ring so mt builds unlock immediately
            cpf_sb = consts.tile([P, CPF_COLS], f32)
            nc.sync.dma_start(cpf_sb, cpf)
            iota = cpf_sb[:, C_IOTA:C_IOTA + P]
            sidf = cpf_sb[:, C_SID:C_SID + BL * KT]
            icnt = cpf_sb[:, C_ICNT:C_ICNT + BL * KT]

            # weights on the gpsimd ring, concurrent with the hidden stream
            w3sb = wpool.tile([P, KH], bf16)
            nc.gpsimd.dma_start(w3sb, w3p)
            w1sb = wpool.tile([P, KH, H], bf16)
            nc.gpsimd.dma_start(w1sb, w1.rearrange("(k p) h -> p k h", p=P))
            w2sb = wpool.tile([P, KH, H], bf16)
            nc.gpsimd.dma_start(w2sb, w2.rearrange("(k p) h -> p k h", p=P))

            # hidden: first two batches per k-tile (pooling starts on the
            # first 0.2 MB), later batches one descriptor each
            hbs = [None] * BL

            def load_hb(b):
                if b < 2:
                    tiles = []
                    for k in range(KT):
                        t_ = hpool.tile([P, H], bf16, tag=f"hb{b}k{k}")
                        nc.sync.dma_start(t_, hid[b, k * P:(k + 1) * P, :])
                        tiles.append(t_)
                    hbs[b] = tiles
                else:
                    hb = hpool.tile(
                        [P, KT, H], bf16, tag=f"hbr{(b - 2) % 4}", name=f"hb{b}"
                    )
                    nc.sync.dma_start(hb, hid[b].rearrange("(k p) h -> p k h", p=P))
                    hbs[b] = hb

            def hsl(b, k, h):
                if b < 2:
                    return hbs[b][k][:, h * P:(h + 1) * P]
                return hbs[b][:, k, h * P:(h + 1) * P]

            for b in range(BL):
                load_hb(b)

            xts = [xpool.tile([P, R], bf16, tag=f"xt{h}", name=f"xt{h}") for h in range(KH)]
            y1s = [xpool.tile([P, R], bf16, tag=f"y1_{m}", name=f"y1_{m}") for m in range(KH)]
            y2s = [xpool.tile([P, R], bf16, tag=f"y2_{m}", name=f"y2_{m}") for m in range(KH)]
            pred = xpool.tile([1, R], f32, tag="pred")
            scratch = xpool.tile([1, 1], f32, tag="scratch")

            # prefetch the gelu activation table during the DMA lead-in
            nc.scalar.activation(scratch, cpf_sb[0:1, 0:1], FT.Gelu)

            def pool(b):
                mts = []
                for k in range(KT):
                    c = b * KT + k
                    mt = mtpool.tile([P, P], bf16, tag="mt")
                    nc.vector.tensor_scalar(
                        mt, iota, sidf[:, c:c + 1], icnt[:, c:c + 1],
                        OP.is_equal, OP.mult,
                    )
                    mts.append(mt)
                for h in range(KH):
                    pt = pp.tile([P, P], f32, tag="pp", name=f"pp{b}_{h}")
                    for k in range(KT):
                        nc.tensor.matmul(
                            pt, lhsT=hsl(b, k, h), rhs=mts[k],
                            start=(k == 0), stop=(k == KT - 1),
                        )
                    nc.vector.tensor_copy(xts[h][:, b * S:(b + 1) * S], pt)

            def fc1(rcp):
                for m in range(KH):
                    pt = pf.tile([P, RW], f32, tag="pf")
                    for half in range(2):
                        rc = 2 * rcp + half
                        for k in range(KH):
                            nc.tensor.matmul(
                                pt[:, half * RC:(half + 1) * RC],
                                lhsT=w1sb[:, k, m * P:(m + 1) * P],
                                rhs=xts[k][:, rc * RC:(rc + 1) * RC],
                                start=(k == 0), stop=(k == KH - 1),
                            )
                    nc.scalar.activation(
                        y1s[m][:, rcp * RW:(rcp + 1) * RW], pt, FT.Gelu,
                        bias=cpf_sb[:, C_B1 + m:C_B1 + m + 1],
                    )

            def fc2(rcp):
                for m in range(KH):
                    pt = pf.tile([P, RW], f32, tag="pf")
                    for k in range(KH):
                        nc.tensor.matmul(
                            pt,
                            lhsT=w2sb[:, k, m * P:(m + 1) * P],
                            rhs=y1s[k][:, rcp * RW:(rcp + 1) * RW],
                            start=(k == 0), stop=(k == KH - 1),
                        )
                    nc.scalar.activation(
                        y2s[m][:, rcp * RW:(rcp + 1) * RW], pt, FT.Gelu,
                        bias=cpf_sb[:, C_B2 + m:C_B2 + m + 1],
                    )

            p3s = [None, None]

            def fc3(rcp):
                pt3 = p3.tile([1, RW], f32, tag=f"p3{rcp}")
                for k in range(KH):
                    nc.tensor.matmul(
                        pt3,
                        lhsT=w3sb[:, k:k + 1],
                        rhs=y2s[k][:, rcp * RW:(rcp + 1) * RW],
                        start=(k == 0), stop=(k == KH - 1),
                    )
                p3s[rcp] = pt3

            for b in range(4):
                pool(b)
            fc1(0)
            for b in range(4, BL):
                pool(b)
            fc1(1)
            fc2(0)
            fc3(0)
            fc2(1)
            # last gelu issued; prefetch the sigmoid table while PE runs fc3
            nc.scalar.activation(scratch, cpf_sb[0:1, 0:1], FT.Sigmoid)
            nc.scalar.activation(
                pred[:, 0:RW], p3s[0], FT.Sigmoid, bias=cpf_sb[0:1, C_B3:C_B3 + 1]
            )
            fc3(1)
            nc.scalar.activation(
                pred[:, RW:2 * RW], p3s[1], FT.Sigmoid,
                bias=cpf_sb[0:1, C_B3:C_B3 + 1],
            )
            nc.sync.dma_start(out.rearrange("b s -> (b s)"), pred[:, :])

    nc.compile()
    return nc


def _get_program():
    if "nc" not in _CACHE:
        _CACHE["nc"] = _build_program()
    return _CACHE["nc"]


def _tok_cols(x):
    """[BL, T] -> [128, BL*KT], column c=b*KT+k holds tokens k*128..k*128+127."""
    return np.transpose(x.reshape(BL, KT, P), (2, 0, 1)).reshape(P, BL * KT)


def make_in_maps(hidden, statements_ids, w1, b1, w2, b2, w3, b3):
    bf = ml_dtypes.bfloat16
    hid_b = np.ascontiguousarray(np.asarray(hidden, np.float32).astype(bf))
    w1b = np.ascontiguousarray(np.asarray(w1, np.float32).astype(bf))
    w2b = np.ascontiguousarray(np.asarray(w2, np.float32).astype(bf))
    w3v = np.asarray(w3, np.float32).reshape(H)
    w3pk = np.ascontiguousarray(w3v.reshape(KH, P).T.astype(bf))  # [128, 6]
    sid = np.asarray(statements_ids, np.int32)
    # per-token inverse segment count (count >= 1 for a token's own sid)
    cnt = (sid[:, :, None] == np.arange(S)[None, None, :]).sum(1)  # [B, S]
    icnt_tok = (1.0 / np.take_along_axis(cnt, sid, 1)).astype(np.float32)

    b1v = np.asarray(b1, np.float32).reshape(KH, P).T  # [128, 6]
    b2v = np.asarray(b2, np.float32).reshape(KH, P).T
    b3v = np.float32(np.asarray(b3).reshape(-1)[0])

    in_maps = []
    for c in range(N_CORES):
        lo, hi = c * BL, (c + 1) * BL
        cpf = np.zeros((P, CPF_COLS), dtype=np.float32)
        cpf[:, C_IOTA:C_IOTA + P] = np.arange(P, dtype=np.float32)[None, :]
        cpf[:, C_SID:C_SID + BL * KT] = _tok_cols(sid[lo:hi].astype(np.float32))
        cpf[:, C_ICNT:C_ICNT + BL * KT] = _tok_cols(icnt_tok[lo:hi])
        cpf[:, C_B1:C_B1 + KH] = b1v
        cpf[:, C_B2:C_B2 + KH] = b2v
        cpf[0, C_B3] = b3v
        in_maps.append(
            {
                "hidden": hid_b[lo:hi],
                "w1": w1b,
                "w2": w2b,
                "w3p": w3pk,
                "cpf": cpf,
            }
        )
    return in_maps


def kernel(hidden, statements_ids, w1, b1, w2, b2, w3, b3, **kwargs):
    nc = _get_program()
    in_maps = make_in_maps(hidden, statements_ids, w1, b1, w2, b2, w3, b3)
    trace = bool(int(os.environ.get("KERNEL_TRACE", "0")))
    res = bass_utils.run_bass_kernel_spmd(
        nc, in_maps, core_ids=list(range(N_CORES)), trace=trace
    )
    _CACHE["last_results"] = res
    out = np.concatenate([res.results[c]["out"] for c in range(N_CORES)], axis=0)
    return out.astype(np.float32)
